# revision 14
# baseline (speedup 1.0000x reference)
"""Non-local (spatial self-attention) denoising block on 8 Trainium2 cores.

Reference math (per sample n, with x:[C,HW], D=C/2):
    t = (W_theta @ x + b_theta) / sqrt(D)      [D, HW]   (1/sqrt(D) folded in)
    p = W_phi   @ x + b_phi                    [D, HW]
    S[q,k] = t[:,q] . p[:,k]
    f = softmax_k(S)
    attn = x @ f.T  (i.e. attn[c,q] = sum_k f[q,k] x[c,k])
    out = x + W_fuse @ attn + b_fuse

Device formulation -- all matmuls fp8(e4m3, TRN flavor: max ±240) with
perf_mode=DoubleRow (2 k-rows per PE cell -> 2x bf16 throughput, 216ns per
N=512 DR matmul measured) and fp32 PSUM accumulation. Scale foldings keep
every fp8 tensor in a good range:
    wcat = 4*[W_theta/sqrt(D); W_phi]  (std ~0.2)   T,P = 4*(t,p) (std ~4.5)
    S'' = T.P = 256*S; e = exp(S''/256 - 3)  (ACT scale+bias; shift cancels
          in softmax, keeps e <= ~60 < 240 for fp8 storage)
    wfu = 16*W_fuse.T;  G'' = x.T@wfu = 16*G'^T  (max ~100, stored fp8)
    Z16 = 16 * ones.T @ e  -- ones memset to 16 undoes the wfu scale; the Z
          sum rides the PE (DoubleRow ones-matmuls accumulated over e-tile
          pairs) instead of a DVE add chain, keeping DVE off the crit path
    out = (G''^T @ e) * (1/Z16) + (x + b_fuse)

Numerics (numpy sim vs reference): L2 rel err 8.6e-3 vs 2e-2 budget.

Software pipeline (the fp8 PE stream is fast enough that a sample's y
matmuls would stall ~5us waiting on its own exps, and the final sample's
combine+DMA was a ~26us pure tail): period s emits
    Z(s-1) | conv(s) | gt(s) | S(s) x-interleaved with y(s-1)+finals(s-1)
so PE always has runnable work while ACT drains the exp chain, the PSUM
pool (4 slots x 2 banks) never holds more than ~4 live tiles, and the
last sample's tail shrinks to Z+y+finals with its output DMAs split
across the (idle by then) sync+scalar rings.

Sharding: data-parallel over batch N=32 -> 4 samples per core on 8 cores.
A ~3.8us burst of junk matmuls pre-warms the PE clock (HAM) while the
first DMAs land.
"""

import numpy as np
import ml_dtypes

import concourse.bass as bass
import concourse.tile as tile
from concourse import bacc, mybir
from concourse import bass_utils

F32 = mybir.dt.float32
BF16 = mybir.dt.bfloat16
FP8 = mybir.dt.float8e4
AF = mybir.ActivationFunctionType
DR = mybir.MatmulPerfMode.DoubleRow

N, C, H, W = 32, 512, 32, 32
D = C // 2
HW = H * W
NCORES = 8
NS = N // NCORES  # samples per core
P = 128
CT = C // P   # 4 c-tiles
KT = HW // P  # 8 hw-tiles
MT_D = (2 * D) // P  # 4 m-tiles of combined theta/phi conv
NQ = HW // 512  # 2 free-dim halves


def _emit(tc):
    nc = tc.nc

    x_q = nc.dram_tensor("x_q", [NS, C, HW], FP8, kind="ExternalInput").ap()
    x_res = nc.dram_tensor("x_res", [NS, C, HW], F32, kind="ExternalInput").ap()
    wcat_t = nc.dram_tensor("wcat_t", [C, 2 * D], FP8, kind="ExternalInput").ap()
    b_cat = nc.dram_tensor("b_cat", [2 * D, 1], F32, kind="ExternalInput").ap()
    wfu_t = nc.dram_tensor("wfu_t", [C, C], FP8, kind="ExternalInput").ap()
    out_d = nc.dram_tensor("out", [NS, C, HW], BF16, kind="ExternalOutput").ap()

    import contextlib
    ctx = contextlib.ExitStack()
    with ctx:
        # ---- constant pools ----
        wpool = ctx.enter_context(tc.tile_pool(name="wpool", bufs=1))
        wcat_sb = wpool.tile([P, CT * 2 * D], FP8)

        # ---- working pools ----
        xq_pool = ctx.enter_context(tc.tile_pool(name="xq", bufs=3))
        xres_pool = ctx.enter_context(tc.tile_pool(name="xres", bufs=2))
        tp_pool = ctx.enter_context(tc.tile_pool(name="tp", bufs=2))
        gt_pool = ctx.enter_context(tc.tile_pool(name="gt", bufs=2))
        e_pool = ctx.enter_context(tc.tile_pool(name="e", bufs=2))
        rz_pool = ctx.enter_context(tc.tile_pool(name="rz", bufs=2))
        fin_pool = ctx.enter_context(tc.tile_pool(name="fin", bufs=3))
        out_pool = ctx.enter_context(tc.tile_pool(name="outp", bufs=3))

        # one PSUM pool: 4 slots x 2 banks = all 8 banks.
        psum_mm = ctx.enter_context(tc.tile_pool(name="psmm", bufs=4, space="PSUM"))

        # HAM pre-warm: ~3.8us of junk matmuls during the initial DMA wait
        # so the real matmuls start at 2.4 GHz instead of 1.2 GHz. Their
        # memsets go on gpsimd, which is ~1us ahead of DVE at kernel start.
        ones_bf = wpool.tile([P, P], BF16)
        nc.gpsimd.memset(ones_bf[:], 1.0)
        warm_rhs = wpool.tile([P, 512], BF16)
        nc.gpsimd.memset(warm_rhs[:], 0.0)
        ones8_sb = wpool.tile([P, 2 * P], FP8)
        nc.vector.memset(ones8_sb[:], 16.0)
        ones8_3 = ones8_sb.rearrange("p (t f) -> p t f", f=P)
        nbias_sb = wpool.tile([P, 1], F32)
        nc.vector.memset(nbias_sb[:], -3.0)
        ps_warm = psum_mm.tile([P, 512], F32, tag="mm", name="ps_warm")
        for w in range(9):
            nc.tensor.matmul(ps_warm[:], ones_bf[:], warm_rhs[:],
                             start=True, stop=True)

        xq_tiles = {}
        xq_tiles[0] = xq_pool.tile([P, CT * HW], FP8, tag="xq", name="xq0")
        for k in range(CT):
            nc.sync.dma_start(
                wcat_sb[:, k * 2 * D:(k + 1) * 2 * D],
                wcat_t.rearrange("(t p) d -> t p d", p=P)[k],
            )
            nc.gpsimd.dma_start(
                xq_tiles[0][:, k * HW:(k + 1) * HW],
                x_q[0].rearrange("(t p) f -> t p f", p=P)[k],
            )

        # remaining constants; bcat (tiny, needed by the first bias adds)
        # goes ahead of the 256KB wfu
        bcat_sb = wpool.tile([P, MT_D], F32)
        nc.sync.dma_start(
            bcat_sb.rearrange("p (t o) -> p t o", o=1),
            b_cat.rearrange("(t p) o -> p t o", p=P),
        )
        wfu_sb = wpool.tile([P, CT * C], FP8)
        nc.sync.dma_start(
            wfu_sb.rearrange("p (t d) -> p t d", d=C),
            wfu_t.rearrange("(t p) d -> p t d", p=P),
        )

        wcat3 = wcat_sb.rearrange("p (t d) -> p t d", d=2 * D)   # [128,4,512]
        wfu3 = wfu_sb.rearrange("p (t d) -> p t d", d=C)         # [128,4,512]

        xres_tiles = {}
        tp_tiles = {}
        gt_tiles = {}
        e_tiles = {}
        rz_tiles = {}
        psy_tiles = {}

        def load_inputs(s):
            if s not in xq_tiles:
                xq_tiles[s] = xq_pool.tile(
                    [P, CT * HW], FP8, tag="xq", name=f"xq{s}"
                )
                nc.sync.dma_start(
                    xq_tiles[s].rearrange("p (t f) -> p t f", f=HW),
                    x_q[s].rearrange("(t p) f -> p t f", p=P),
                )

        def conv_phase(s):
            # combined theta/phi 1x1 conv: TP = wcat.T @ x + 4b
            # tp chunks m=0,1 -> T [256, HW]; m=2,3 -> P(phi)
            xq3 = xq_tiles[s].rearrange("p (t f) -> p t f", f=HW)
            tp_tiles[s] = tp_pool.tile([P, MT_D * HW], FP8, tag="tp",
                                       name=f"tp{s}")
            tp_sb = tp_tiles[s]

            def conv_mm(ps, m, k2):
                for nq in range(NQ):
                    nc.tensor.matmul(
                        ps[:, nq * 512:(nq + 1) * 512],
                        wcat3[:, 2 * k2:2 * k2 + 2, m * P:(m + 1) * P],
                        xq3[:, 2 * k2:2 * k2 + 2, nq * 512:nq * 512 + 512],
                        start=(k2 == 0),
                        stop=(k2 == 1),
                        perf_mode=DR,
                    )

            def conv_copy(ps, m):
                # bias add + fp8 cast, split DVE(h0)/ACT(h1): a serial DVE
                # chain (4x1.27us) held the conv PSUM slots past the point
                # where the gt matmuls needed them (measured 0.83us PE gap
                # per period); two engines drain slots at PE's fill rate
                nc.vector.tensor_scalar_add(
                    tp_sb[:, m * HW:m * HW + 512], ps[:, 0:512],
                    bcat_sb[:, m:m + 1],
                )
                nc.scalar.add(
                    tp_sb[:, m * HW + 512:(m + 1) * HW], ps[:, 512:HW],
                    bcat_sb[:, m:m + 1],
                )

            if s == 0:
                # k2-outer so each arriving x-chunk pair feeds 8 matmuls
                # immediately instead of stalling m0 on chunks in flight
                ps_cvs = [
                    psum_mm.tile([P, HW], F32, tag="mm", name=f"ps_cv0_{m}")
                    for m in range(MT_D)
                ]
                for k2 in range(2):
                    for m in range(MT_D):
                        conv_mm(ps_cvs[m], m, k2)
                for m in range(MT_D):
                    conv_copy(ps_cvs[m], m)
            else:
                for m in range(MT_D):
                    ps_cv = psum_mm.tile(
                        [P, HW], F32, tag="mm", name=f"ps_cv{s}_{m}"
                    )
                    for k2 in range(2):
                        conv_mm(ps_cv, m, k2)
                    conv_copy(ps_cv, m)

        def gt_phase(s):
            # G'' = x.T @ wfu : [HW, C] = 16*G'^T, fused-values
            xq3 = xq_tiles[s].rearrange("p (t f) -> p t f", f=HW)
            gt_tiles[s] = gt_pool.tile([P, KT * C], FP8, tag="gt",
                                       name=f"gt{s}")
            for m in range(KT):
                ps_g = psum_mm.tile([P, C], F32, tag="mm", name=f"ps_g{s}_{m}")
                for k2 in range(2):
                    nc.tensor.matmul(
                        ps_g[:],
                        xq3[:, 2 * k2:2 * k2 + 2, m * P:(m + 1) * P],
                        wfu3[:, 2 * k2:2 * k2 + 2, :],
                        start=(k2 == 0),
                        stop=(k2 == 1),
                        perf_mode=DR,
                    )
                nc.scalar.activation(
                    gt_tiles[s][:, m * C:(m + 1) * C], ps_g[:], AF.Copy,
                )

        def s_tile(s, m):
            # S'' = P.T @ T (x256) for key-tile m; e = exp(S''/256 - 3)
            tp3 = tp_tiles[s].rearrange("p (t f) -> p t f", f=HW)
            ps_s = psum_mm.tile([P, HW], F32, tag="mm", name=f"ps_s{s}_{m}")
            for nq in range(NQ):
                nc.tensor.matmul(
                    ps_s[:, nq * 512:(nq + 1) * 512],
                    tp3[:, 2:4, m * P:(m + 1) * P],
                    tp3[:, 0:2, nq * 512:nq * 512 + 512],
                    start=True,
                    stop=True,
                    perf_mode=DR,
                )
            nc.scalar.activation(
                e_tiles[s][:, m * HW:(m + 1) * HW], ps_s[:], AF.Exp,
                bias=nbias_sb[:], scale=1.0 / 256.0,
            )

        def z_phase(s):
            # Z16[q] = 16 * sum_k e[k,q], summed over partitions AND broadcast
            # to all 128 of them by the ones(=16) matmuls; accumulated on PE
            # over the 4 e-tile pairs (DoubleRow), not a DVE add chain. Runs
            # at the NEXT period's head, when e(s) is long done -> no stall.
            e3 = e_tiles[s].rearrange("p (t f) -> p t f", f=HW)
            ps_z = psum_mm.tile([P, HW], F32, tag="mm", name=f"ps_z{s}")
            for nq in range(NQ):
                for k2 in range(KT // 2):
                    nc.tensor.matmul(
                        ps_z[:, nq * 512:(nq + 1) * 512],
                        ones8_3[:, 0:2, :],
                        e3[:, 2 * k2:2 * k2 + 2, nq * 512:nq * 512 + 512],
                        start=(k2 == 0),
                        stop=(k2 == KT // 2 - 1),
                        perf_mode=DR,
                    )
            rz_tiles[s] = rz_pool.tile([P, HW], F32, tag="rz", name=f"rz{s}")
            nc.vector.reciprocal_approx_fast(
                out=rz_tiles[s][:, 0:512], in_=ps_z[:, 0:512])
            nc.vector.reciprocal_approx_fast(
                out=rz_tiles[s][:, 512:HW], in_=ps_z[:, 512:HW])

        def y_tile(s, m):
            # y = G''^T @ e : [C, HW] unnormalized attn+conv (x16)
            gt3 = gt_tiles[s].rearrange("p (t f) -> p t f", f=C)
            e3 = e_tiles[s].rearrange("p (t f) -> p t f", f=HW)
            ps_y = psum_mm.tile([P, HW], F32, tag="mm", name=f"ps_y{s}_{m}")
            for k2 in range(KT // 2):
                for nq in range(NQ):
                    nc.tensor.matmul(
                        ps_y[:, nq * 512:(nq + 1) * 512],
                        gt3[:, 2 * k2:2 * k2 + 2, m * P:(m + 1) * P],
                        e3[:, 2 * k2:2 * k2 + 2, nq * 512:nq * 512 + 512],
                        start=(k2 == 0),
                        stop=(k2 == KT // 2 - 1),
                        perf_mode=DR,
                    )
            psy_tiles[(s, m)] = ps_y

        def fin_tile(s, m, last):
            # final combine in 512-halves: mul (psum, must be DVE) then the
            # residual add. The mul also frees y(s,m)'s PSUM slot for the
            # next allocation in the interleave.
            t1 = fin_pool.tile([P, HW], F32, tag="fin", name=f"t1_{s}_{m}")
            o_sb = out_pool.tile([P, HW], BF16, tag="o", name=f"o_{s}_{m}")
            ps_y = psy_tiles.pop((s, m))
            xres_sb = xres_tiles[s]
            for h in range(2):
                hs = slice(h * 512, (h + 1) * 512)
                nc.vector.tensor_mul(t1[:, hs], ps_y[:, hs], rz_tiles[s][:, hs])
                # bf16 outputs halve the out-DMA bytes (+0.1% rel err). The
                # h0 adds ride the otherwise-idle gpsimd (slow but parallel)
                # for every sample: a DVE-only fin chain backlogged into the
                # epilogue and made the last sample's tail trail PE by ~6us.
                add_eng = nc.gpsimd if h == 0 else nc.vector
                add_eng.tensor_add(
                    o_sb[:, hs], t1[:, hs],
                    xres_sb[:, m * HW + h * 512: m * HW + h * 512 + 512],
                )
                if last:
                    # tail DMAs split across the idle sync+scalar rings
                    (nc.sync if h == 0 else nc.scalar).dma_start(
                        out_d[s].rearrange("(t p) f -> t p f", p=P)[m][:, hs],
                        o_sb[:, hs],
                    )
            if not last:
                # one descriptor per m-tile (not per half): halves the
                # gpsimd queue's ~660ns-per-descriptor issue cost
                nc.gpsimd.dma_start(
                    out_d[s].rearrange("(t p) f -> t p f", p=P)[m],
                    o_sb[:],
                )

        for s in range(NS):
            load_inputs(s + 1) if s + 1 < NS else None
            xres_tiles[s] = xres_pool.tile([P, CT * HW], F32, tag="xres",
                                           name=f"xres{s}")
            nc.sync.dma_start(
                xres_tiles[s].rearrange("p (t f) -> p t f", f=HW),
                x_res[s].rearrange("(t p) f -> p t f", p=P),
            )
            conv_phase(s)
            if s > 0:
                # after conv: by then the previous sample's last exp is
                # certainly done, so the Z accumulation never stalls PE
                z_phase(s - 1)
            gt_phase(s)
            e_tiles[s] = e_pool.tile([P, KT * HW], FP8, tag="e", name=f"e{s}")
            if s == 0:
                for m in range(KT):
                    s_tile(s, m)
            else:
                # interleave: S(s) pairs alternate with y(s-1) m-tiles and
                # finals(s-1), so PE never stalls on the exp-gated PSUM
                # slot rotation and y's slots are freed by the fin muls.
                # all fins land before the last S-tile: at the period
                # boundary only S7's exp-pending slot is live, so the next
                # conv's PSUM allocations never wait on this period's DVE
                s_tile(s, 0)
                s_tile(s, 1)
                y_tile(s - 1, 0)
                s_tile(s, 2)
                fin_tile(s - 1, 0, last=False)
                s_tile(s, 3)
                y_tile(s - 1, 1)
                s_tile(s, 4)
                fin_tile(s - 1, 1, last=False)
                s_tile(s, 5)
                y_tile(s - 1, 2)
                fin_tile(s - 1, 2, last=False)
                s_tile(s, 6)
                y_tile(s - 1, 3)
                fin_tile(s - 1, 3, last=False)
                s_tile(s, 7)
            # free the previous sample's big tiles
            if s > 0:
                del e_tiles[s - 1], gt_tiles[s - 1], tp_tiles[s - 1]
                del xres_tiles[s - 1], rz_tiles[s - 1]

        # epilogue: last sample's attention tail
        sl = NS - 1
        z_phase(sl)
        for m in range(CT):
            y_tile(sl, m)
            fin_tile(sl, m, last=True)


_CACHE = {}


def _build():
    if "nc" not in _CACHE:
        nc = bacc.Bacc("TRN2", target_bir_lowering=False, debug=False)
        with tile.TileContext(nc) as tc:
            _emit(tc)
        nc.compile()
        _CACHE["nc"] = nc
    return _CACHE["nc"]


def _prep_in_maps(x, W_theta, b_theta, W_phi, b_phi, W_fuse, b_fuse):
    f8 = ml_dtypes.float8_e4m3
    scale = np.float32(1.0 / np.sqrt(np.float32(D)))

    def q8(a):
        return np.clip(a, -240.0, 240.0).astype(f8)

    xf = np.ascontiguousarray(x.reshape(N, C, HW).astype(np.float32))
    x_q = q8(xf)
    x_res = xf + b_fuse.astype(np.float32)[None, :, None]
    # wcat = 4*[W_theta/sqrt(D); W_phi] -> S arrives x256, undone in the exp
    wcat_t = q8(np.ascontiguousarray(
        np.concatenate([W_theta.astype(np.float32) * scale * 64.0,
                        W_phi.astype(np.float32) * 4.0], axis=0).T
    ))
    b_cat = np.concatenate([b_theta.astype(np.float32) * scale * 64.0,
                            b_phi.astype(np.float32) * 4.0]).reshape(2 * D, 1)
    wfu_t = q8(np.ascontiguousarray(W_fuse.astype(np.float32).T) * 16.0)

    in_maps = []
    for c in range(NCORES):
        sl = slice(c * NS, (c + 1) * NS)
        in_maps.append({
            "x_q": np.ascontiguousarray(x_q[sl]),
            "x_res": np.ascontiguousarray(x_res[sl]),
            "wcat_t": wcat_t,
            "b_cat": b_cat.astype(np.float32),
            "wfu_t": wfu_t,
        })
    return in_maps


def _run(inputs, trace=False, **kw):
    nc = _build()
    in_maps = _prep_in_maps(**inputs)
    res = bass_utils.run_bass_kernel_spmd(
        nc, in_maps, core_ids=list(range(NCORES)), trace=trace, **kw
    )
    out = np.concatenate(
        [res.results[c]["out"].astype(np.float32) for c in range(NCORES)],
        axis=0,
    )
    return out.reshape(N, C, H, W), res


def kernel(**inputs):
    inputs = {k: np.asarray(v) for k, v in inputs.items()}
    out, _ = _run(inputs, trace=False)
    return out


# revision 15
# speedup vs baseline: 1.1464x; 1.1464x over previous
"""Non-local (spatial self-attention) denoising block on 8 Trainium2 cores.

Reference math (per sample n, with x:[C,HW], D=C/2):
    t = (W_theta @ x + b_theta) / sqrt(D)      [D, HW]   (1/sqrt(D) folded in)
    p = W_phi   @ x + b_phi                    [D, HW]
    S[q,k] = t[:,q] . p[:,k]
    f = softmax_k(S)
    attn = x @ f.T  (i.e. attn[c,q] = sum_k f[q,k] x[c,k])
    out = x + W_fuse @ attn + b_fuse

Device formulation -- all matmuls fp8(e4m3, TRN flavor: max ±240) with
perf_mode=DoubleRow (2 k-rows per PE cell -> 2x bf16 throughput, 216ns per
N=512 DR matmul measured) and fp32 PSUM accumulation. Scale foldings keep
every fp8 tensor in a good range:
    wcat = 4*[W_theta/sqrt(D); W_phi]  (std ~0.2)   T,P = 4*(t,p) (std ~4.5)
    S'' = T.P = 256*S; e = exp(S''/256 - 3)  (ACT scale+bias; shift cancels
          in softmax, keeps e <= ~60 < 240 for fp8 storage)
    wfu = 16*W_fuse.T;  G'' = x.T@wfu = 16*G'^T  (max ~100, stored fp8)
    Z16 = 16 * ones.T @ e  -- ones memset to 16 undoes the wfu scale; the Z
          sum rides the PE (DoubleRow ones-matmuls accumulated over e-tile
          pairs) instead of a DVE add chain, keeping DVE off the crit path
    out = (G''^T @ e) * (1/Z16) + (x + b_fuse)

Numerics (numpy sim vs reference): L2 rel err 8.6e-3 vs 2e-2 budget.

Software pipeline (the fp8 PE stream is fast enough that a sample's y
matmuls would stall ~5us waiting on its own exps, and the final sample's
combine+DMA was a ~26us pure tail): period s emits
    Z(s-1) | conv(s) | gt(s) | S(s) x-interleaved with y(s-1)+finals(s-1)
so PE always has runnable work while ACT drains the exp chain, the PSUM
pool (4 slots x 2 banks) never holds more than ~4 live tiles, and the
last sample's tail shrinks to Z+y+finals with its output DMAs split
across the (idle by then) sync+scalar rings.

Sharding: data-parallel over batch N=32 -> 4 samples per core on 8 cores.
A ~3.8us burst of junk matmuls pre-warms the PE clock (HAM) while the
first DMAs land.
"""

import numpy as np
import ml_dtypes

import concourse.bass as bass
import concourse.tile as tile
from concourse import bacc, mybir
from concourse import bass_utils

F32 = mybir.dt.float32
BF16 = mybir.dt.bfloat16
FP8 = mybir.dt.float8e4
AF = mybir.ActivationFunctionType
DR = mybir.MatmulPerfMode.DoubleRow

N, C, H, W = 32, 512, 32, 32
D = C // 2
HW = H * W
NCORES = 8
NS = N // NCORES  # samples per core
P = 128
CT = C // P   # 4 c-tiles
KT = HW // P  # 8 hw-tiles
MT_D = (2 * D) // P  # 4 m-tiles of combined theta/phi conv
NQ = HW // 512  # 2 free-dim halves


def _emit(tc):
    nc = tc.nc

    x_q = nc.dram_tensor("x_q", [NS, C, HW], FP8, kind="ExternalInput").ap()
    x_res = nc.dram_tensor("x_res", [NS, C, HW], F32, kind="ExternalInput").ap()
    wcat_t = nc.dram_tensor("wcat_t", [C, 2 * D], FP8, kind="ExternalInput").ap()
    b_cat = nc.dram_tensor("b_cat", [2 * D, 1], F32, kind="ExternalInput").ap()
    wfu_t = nc.dram_tensor("wfu_t", [C, C], FP8, kind="ExternalInput").ap()
    out_d = nc.dram_tensor("out", [NS, C, HW], BF16, kind="ExternalOutput").ap()

    import contextlib
    ctx = contextlib.ExitStack()
    with ctx:
        # ---- constant pools ----
        wpool = ctx.enter_context(tc.tile_pool(name="wpool", bufs=1))
        wcat_sb = wpool.tile([P, CT * 2 * D], FP8)

        # ---- working pools ----
        xq_pool = ctx.enter_context(tc.tile_pool(name="xq", bufs=3))
        xres_pool = ctx.enter_context(tc.tile_pool(name="xres", bufs=2))
        tp_pool = ctx.enter_context(tc.tile_pool(name="tp", bufs=2))
        gt_pool = ctx.enter_context(tc.tile_pool(name="gt", bufs=2))
        e_pool = ctx.enter_context(tc.tile_pool(name="e", bufs=2))
        rz_pool = ctx.enter_context(tc.tile_pool(name="rz", bufs=2))
        fin_pool = ctx.enter_context(tc.tile_pool(name="fin", bufs=3))
        out_pool = ctx.enter_context(tc.tile_pool(name="outp", bufs=3))

        # one PSUM pool: 4 slots x 2 banks = all 8 banks.
        psum_mm = ctx.enter_context(tc.tile_pool(name="psmm", bufs=4, space="PSUM"))

        # HAM pre-warm: ~3.8us of junk matmuls during the initial DMA wait
        # so the real matmuls start at 2.4 GHz instead of 1.2 GHz. Their
        # memsets go on gpsimd, which is ~1us ahead of DVE at kernel start.
        ones_bf = wpool.tile([P, P], BF16)
        nc.gpsimd.memset(ones_bf[:], 1.0)
        warm_rhs = wpool.tile([P, 512], BF16)
        nc.gpsimd.memset(warm_rhs[:], 0.0)
        ones8_sb = wpool.tile([P, 2 * P], FP8)
        nc.vector.memset(ones8_sb[:], 16.0)
        ones8_3 = ones8_sb.rearrange("p (t f) -> p t f", f=P)
        nbias_sb = wpool.tile([P, 1], F32)
        nc.vector.memset(nbias_sb[:], -3.0)
        ps_warm = psum_mm.tile([P, 512], F32, tag="mm", name="ps_warm")
        for w in range(9):
            nc.tensor.matmul(ps_warm[:], ones_bf[:], warm_rhs[:],
                             start=True, stop=True)

        xq_tiles = {}
        xq_tiles[0] = xq_pool.tile([P, CT * HW], FP8, tag="xq", name="xq0")
        for k in range(CT):
            nc.sync.dma_start(
                wcat_sb[:, k * 2 * D:(k + 1) * 2 * D],
                wcat_t.rearrange("(t p) d -> t p d", p=P)[k],
            )
            nc.gpsimd.dma_start(
                xq_tiles[0][:, k * HW:(k + 1) * HW],
                x_q[0].rearrange("(t p) f -> t p f", p=P)[k],
            )

        # remaining constants; bcat (tiny, needed by the first bias adds)
        # goes ahead of the 256KB wfu
        bcat_sb = wpool.tile([P, MT_D], F32)
        nc.sync.dma_start(
            bcat_sb.rearrange("p (t o) -> p t o", o=1),
            b_cat.rearrange("(t p) o -> p t o", p=P),
        )
        wfu_sb = wpool.tile([P, CT * C], FP8)
        nc.sync.dma_start(
            wfu_sb.rearrange("p (t d) -> p t d", d=C),
            wfu_t.rearrange("(t p) d -> p t d", p=P),
        )

        wcat3 = wcat_sb.rearrange("p (t d) -> p t d", d=2 * D)   # [128,4,512]
        wfu3 = wfu_sb.rearrange("p (t d) -> p t d", d=C)         # [128,4,512]

        xres_tiles = {}
        tp_tiles = {}
        gt_tiles = {}
        e_tiles = {}
        rz_tiles = {}
        psy_tiles = {}

        def load_inputs(s):
            if s not in xq_tiles:
                xq_tiles[s] = xq_pool.tile(
                    [P, CT * HW], FP8, tag="xq", name=f"xq{s}"
                )
                nc.sync.dma_start(
                    xq_tiles[s].rearrange("p (t f) -> p t f", f=HW),
                    x_q[s].rearrange("(t p) f -> p t f", p=P),
                )

        def conv_phase(s):
            # combined theta/phi 1x1 conv: TP = wcat.T @ x + 4b
            # tp chunks m=0,1 -> T [256, HW]; m=2,3 -> P(phi)
            xq3 = xq_tiles[s].rearrange("p (t f) -> p t f", f=HW)
            tp_tiles[s] = tp_pool.tile([P, MT_D * HW], FP8, tag="tp",
                                       name=f"tp{s}")
            tp_sb = tp_tiles[s]

            def conv_mm(ps, m, k2):
                for nq in range(NQ):
                    nc.tensor.matmul(
                        ps[:, nq * 512:(nq + 1) * 512],
                        wcat3[:, 2 * k2:2 * k2 + 2, m * P:(m + 1) * P],
                        xq3[:, 2 * k2:2 * k2 + 2, nq * 512:nq * 512 + 512],
                        start=(k2 == 0),
                        stop=(k2 == 1),
                        perf_mode=DR,
                    )

            def conv_copy(ps, m):
                # bias add + fp8 cast, split DVE(h0)/ACT(h1): a serial DVE
                # chain (4x1.27us) held the conv PSUM slots past the point
                # where the gt matmuls needed them (measured 0.83us PE gap
                # per period); two engines drain slots at PE's fill rate
                nc.vector.tensor_scalar_add(
                    tp_sb[:, m * HW:m * HW + 512], ps[:, 0:512],
                    bcat_sb[:, m:m + 1],
                )
                nc.scalar.add(
                    tp_sb[:, m * HW + 512:(m + 1) * HW], ps[:, 512:HW],
                    bcat_sb[:, m:m + 1],
                )

            if s == 0:
                # k2-outer so each arriving x-chunk pair feeds 8 matmuls
                # immediately instead of stalling m0 on chunks in flight
                ps_cvs = [
                    psum_mm.tile([P, HW], F32, tag="mm", name=f"ps_cv0_{m}")
                    for m in range(MT_D)
                ]
                for k2 in range(2):
                    for m in range(MT_D):
                        conv_mm(ps_cvs[m], m, k2)
                for m in range(MT_D):
                    conv_copy(ps_cvs[m], m)
            else:
                for m in range(MT_D):
                    ps_cv = psum_mm.tile(
                        [P, HW], F32, tag="mm", name=f"ps_cv{s}_{m}"
                    )
                    for k2 in range(2):
                        conv_mm(ps_cv, m, k2)
                    conv_copy(ps_cv, m)

        def gt_phase(s):
            # G'' = x.T @ wfu : [HW, C] = 16*G'^T, fused-values
            xq3 = xq_tiles[s].rearrange("p (t f) -> p t f", f=HW)
            gt_tiles[s] = gt_pool.tile([P, KT * C], FP8, tag="gt",
                                       name=f"gt{s}")
            for m in range(KT):
                ps_g = psum_mm.tile([P, C], F32, tag="mm", name=f"ps_g{s}_{m}")
                for k2 in range(2):
                    nc.tensor.matmul(
                        ps_g[:],
                        xq3[:, 2 * k2:2 * k2 + 2, m * P:(m + 1) * P],
                        wfu3[:, 2 * k2:2 * k2 + 2, :],
                        start=(k2 == 0),
                        stop=(k2 == 1),
                        perf_mode=DR,
                    )
                nc.scalar.activation(
                    gt_tiles[s][:, m * C:(m + 1) * C], ps_g[:], AF.Copy,
                )

        def s_tile(s, m):
            # S'' = P.T @ T (x256) for key-tile m; e = exp(S''/256 - 3)
            tp3 = tp_tiles[s].rearrange("p (t f) -> p t f", f=HW)
            ps_s = psum_mm.tile([P, HW], F32, tag="mm", name=f"ps_s{s}_{m}")
            for nq in range(NQ):
                nc.tensor.matmul(
                    ps_s[:, nq * 512:(nq + 1) * 512],
                    tp3[:, 2:4, m * P:(m + 1) * P],
                    tp3[:, 0:2, nq * 512:nq * 512 + 512],
                    start=True,
                    stop=True,
                    perf_mode=DR,
                )
            nc.scalar.activation(
                e_tiles[s][:, m * HW:(m + 1) * HW], ps_s[:], AF.Exp,
                bias=nbias_sb[:], scale=1.0 / 256.0,
            )

        def z_phase(s):
            # Z16[q] = 16 * sum_k e[k,q], summed over partitions AND broadcast
            # to all 128 of them by the ones(=16) matmuls; accumulated on PE
            # over the 4 e-tile pairs (DoubleRow), not a DVE add chain. Runs
            # at the NEXT period's head, when e(s) is long done -> no stall.
            e3 = e_tiles[s].rearrange("p (t f) -> p t f", f=HW)
            ps_z = psum_mm.tile([P, HW], F32, tag="mm", name=f"ps_z{s}")
            for nq in range(NQ):
                for k2 in range(KT // 2):
                    nc.tensor.matmul(
                        ps_z[:, nq * 512:(nq + 1) * 512],
                        ones8_3[:, 0:2, :],
                        e3[:, 2 * k2:2 * k2 + 2, nq * 512:nq * 512 + 512],
                        start=(k2 == 0),
                        stop=(k2 == KT // 2 - 1),
                        perf_mode=DR,
                    )
            rz_tiles[s] = rz_pool.tile([P, HW], F32, tag="rz", name=f"rz{s}")
            nc.vector.reciprocal_approx_fast(
                out=rz_tiles[s][:, 0:512], in_=ps_z[:, 0:512])
            nc.vector.reciprocal_approx_fast(
                out=rz_tiles[s][:, 512:HW], in_=ps_z[:, 512:HW])

        def y_tile(s, m):
            # y = G''^T @ e : [C, HW] unnormalized attn+conv (x16)
            gt3 = gt_tiles[s].rearrange("p (t f) -> p t f", f=C)
            e3 = e_tiles[s].rearrange("p (t f) -> p t f", f=HW)
            ps_y = psum_mm.tile([P, HW], F32, tag="mm", name=f"ps_y{s}_{m}")
            for k2 in range(KT // 2):
                for nq in range(NQ):
                    nc.tensor.matmul(
                        ps_y[:, nq * 512:(nq + 1) * 512],
                        gt3[:, 2 * k2:2 * k2 + 2, m * P:(m + 1) * P],
                        e3[:, 2 * k2:2 * k2 + 2, nq * 512:nq * 512 + 512],
                        start=(k2 == 0),
                        stop=(k2 == KT // 2 - 1),
                        perf_mode=DR,
                    )
            psy_tiles[(s, m)] = ps_y

        def fin_tile(s, m, last):
            # final combine in 512-halves: mul (psum, must be DVE) then the
            # residual add. The mul also frees y(s,m)'s PSUM slot for the
            # next allocation in the interleave.
            t1 = fin_pool.tile([P, HW], F32, tag="fin", name=f"t1_{s}_{m}")
            o_sb = out_pool.tile([P, HW], BF16, tag="o", name=f"o_{s}_{m}")
            ps_y = psy_tiles.pop((s, m))
            xres_sb = xres_tiles[s]
            # bf16 outputs halve the out-DMA bytes (+0.1% rel err). The two
            # tail samples' fins (they run in the last period + epilogue)
            # offload h0 adds to the otherwise-idle gpsimd so DVE enters the
            # epilogue without a backlog; their DMAs go to rings whose queues
            # never wait on another engine (queue-coupling a gpsimd DMA to a
            # DVE add measured as a 16us regression). Mid samples stay DVE +
            # gpsimd ring (sync carries their 2.5MB/sample of inputs).
            tail2 = s >= NS - 2
            for h in range(2):
                hs = slice(h * 512, (h + 1) * 512)
                nc.vector.tensor_mul(t1[:, hs], ps_y[:, hs], rz_tiles[s][:, hs])
                add_eng = nc.gpsimd if (tail2 and h == 0) else nc.vector
                add_eng.tensor_add(
                    o_sb[:, hs], t1[:, hs],
                    xres_sb[:, m * HW + h * 512: m * HW + h * 512 + 512],
                )
                if last:
                    dma_eng = nc.sync if h == 0 else nc.scalar
                elif tail2:
                    dma_eng = nc.sync
                else:
                    dma_eng = nc.gpsimd
                dma_eng.dma_start(
                    out_d[s].rearrange("(t p) f -> t p f", p=P)[m][:, hs],
                    o_sb[:, hs],
                )

        for s in range(NS):
            load_inputs(s + 1) if s + 1 < NS else None
            xres_tiles[s] = xres_pool.tile([P, CT * HW], F32, tag="xres",
                                           name=f"xres{s}")
            nc.sync.dma_start(
                xres_tiles[s].rearrange("p (t f) -> p t f", f=HW),
                x_res[s].rearrange("(t p) f -> p t f", p=P),
            )
            conv_phase(s)
            if s > 0:
                # after conv: by then the previous sample's last exp is
                # certainly done, so the Z accumulation never stalls PE
                z_phase(s - 1)
            gt_phase(s)
            e_tiles[s] = e_pool.tile([P, KT * HW], FP8, tag="e", name=f"e{s}")
            if s == 0:
                for m in range(KT):
                    s_tile(s, m)
            else:
                # interleave: S(s) pairs alternate with y(s-1) m-tiles and
                # finals(s-1), so PE never stalls on the exp-gated PSUM
                # slot rotation and y's slots are freed by the fin muls.
                # all fins land before the last S-tile: at the period
                # boundary only S7's exp-pending slot is live, so the next
                # conv's PSUM allocations never wait on this period's DVE
                s_tile(s, 0)
                s_tile(s, 1)
                y_tile(s - 1, 0)
                s_tile(s, 2)
                fin_tile(s - 1, 0, last=False)
                s_tile(s, 3)
                y_tile(s - 1, 1)
                s_tile(s, 4)
                fin_tile(s - 1, 1, last=False)
                s_tile(s, 5)
                y_tile(s - 1, 2)
                fin_tile(s - 1, 2, last=False)
                s_tile(s, 6)
                y_tile(s - 1, 3)
                fin_tile(s - 1, 3, last=False)
                s_tile(s, 7)
            # free the previous sample's big tiles
            if s > 0:
                del e_tiles[s - 1], gt_tiles[s - 1], tp_tiles[s - 1]
                del xres_tiles[s - 1], rz_tiles[s - 1]

        # epilogue: last sample's attention tail
        sl = NS - 1
        z_phase(sl)
        for m in range(CT):
            y_tile(sl, m)
            fin_tile(sl, m, last=True)


_CACHE = {}


def _build():
    if "nc" not in _CACHE:
        nc = bacc.Bacc("TRN2", target_bir_lowering=False, debug=False)
        with tile.TileContext(nc) as tc:
            _emit(tc)
        nc.compile()
        _CACHE["nc"] = nc
    return _CACHE["nc"]


def _prep_in_maps(x, W_theta, b_theta, W_phi, b_phi, W_fuse, b_fuse):
    f8 = ml_dtypes.float8_e4m3
    scale = np.float32(1.0 / np.sqrt(np.float32(D)))

    def q8(a):
        return np.clip(a, -240.0, 240.0).astype(f8)

    xf = np.ascontiguousarray(x.reshape(N, C, HW).astype(np.float32))
    x_q = q8(xf)
    x_res = xf + b_fuse.astype(np.float32)[None, :, None]
    # wcat = 4*[W_theta/sqrt(D); W_phi] -> S arrives x256, undone in the exp
    wcat_t = q8(np.ascontiguousarray(
        np.concatenate([W_theta.astype(np.float32) * scale * 64.0,
                        W_phi.astype(np.float32) * 4.0], axis=0).T
    ))
    b_cat = np.concatenate([b_theta.astype(np.float32) * scale * 64.0,
                            b_phi.astype(np.float32) * 4.0]).reshape(2 * D, 1)
    wfu_t = q8(np.ascontiguousarray(W_fuse.astype(np.float32).T) * 16.0)

    in_maps = []
    for c in range(NCORES):
        sl = slice(c * NS, (c + 1) * NS)
        in_maps.append({
            "x_q": np.ascontiguousarray(x_q[sl]),
            "x_res": np.ascontiguousarray(x_res[sl]),
            "wcat_t": wcat_t,
            "b_cat": b_cat.astype(np.float32),
            "wfu_t": wfu_t,
        })
    return in_maps


def _run(inputs, trace=False, **kw):
    nc = _build()
    in_maps = _prep_in_maps(**inputs)
    res = bass_utils.run_bass_kernel_spmd(
        nc, in_maps, core_ids=list(range(NCORES)), trace=trace, **kw
    )
    out = np.concatenate(
        [res.results[c]["out"].astype(np.float32) for c in range(NCORES)],
        axis=0,
    )
    return out.reshape(N, C, H, W), res


def kernel(**inputs):
    inputs = {k: np.asarray(v) for k, v in inputs.items()}
    out, _ = _run(inputs, trace=False)
    return out
